# revision 52
# baseline (speedup 1.0000x reference)
"""NCE classifier scores kernel for Trainium2 (8 NeuronCores, SPMD).

scores = -(||q||^2 + ||p||^2 - 2 q.p) / T  for q = x[:8192], p = x[8192:].

Sharding: 2D grid (4 query shards x 2 proto shards). Each core computes a
[2048, 4096] output slab, minimizing per-core HBM input traffic.

Host-side staging (data marshalling only — all FLOPs stay on device):
  - operands cast to fp8e4 scaled by s = sqrt(2/T), pre-transposed into
    the [128(d), 2(k-group), n] DoubleRow matmul layout,
  - f16 copies of the natural [row, d] layout for the on-device norms,
  - output comes back f16 and is upcast to f32 on the host.

Per-core device kernel:
  - fp8 DoubleRow matmuls (K=256/instr, 2x bf16 rate) accumulate
    (2/T) q.p into PSUM: 512 matmuls of [128q x 512p], 4 PSUM banks deep.
  - ScalarE Square(scale=sqrt(1/T))+accum on the f16 naturals gives
    ||q||^2/T (per-partition scalar) and ||p||^2/T (per-chunk row).
  - ||p||^2/T rows are gathered to [1, 512] by tiny DMAs and broadcast
    across partitions by GpSimd partition_broadcast.
  - VectorE scalar_tensor_tensor applies both rank-1 corrections in one
    op per tile and writes f16: out = (psum - ||q||^2/T) - ||p||^2/T.
"""

import os
import sys

import numpy as np

NUM_BATCH = 8192
NUM_PROTO = 8192
DIM = 1024
N_CORES = 8
QSHARDS = 4
PSHARDS = 2
QPC = NUM_BATCH // QSHARDS  # 2048 queries per core
PPC = NUM_PROTO // PSHARDS  # 4096 protos per core
P = 128  # partitions
CH = 512  # proto chunk width (= one PSUM bank of f32)
NCH = PPC // CH  # 8 chunks
KT = DIM // P  # 8 contraction tiles of 128
KP = KT // 2  # 4 DoubleRow k-pair tiles
NQT = QPC // P  # 16 query row-groups per core


def _install_axon_hooks_shim():
    """Provide antenv.axon_hooks (NTFF profiling hook) if the image lacks it."""
    try:
        import antenv.axon_hooks  # noqa: F401

        return
    except ImportError:
        pass
    import contextlib
    import ctypes
    import types

    mod = types.ModuleType("antenv.axon_hooks")
    _state = {"hook": None}
    mod.set_axon_ntff_profile_hook = lambda h: _state.__setitem__("hook", h)
    mod.get_axon_ntff_profile_hook = lambda: _state["hook"]
    sys.modules["antenv.axon_hooks"] = mod
    try:
        import antenv

        antenv.axon_hooks = mod
    except ImportError:
        pass
    so_path = "/opt/axon/libaxon_pjrt.so"
    if not os.path.exists(so_path):
        return
    try:
        lib = ctypes.CDLL(so_path)
        if not hasattr(lib, "axon_start_nrt_profile"):
            return
        lib.axon_start_nrt_profile.argtypes = [
            ctypes.POINTER(ctypes.c_int64),
            ctypes.c_size_t,
        ]
        lib.axon_start_nrt_profile.restype = ctypes.c_int64
        lib.axon_stop_nrt_profile.argtypes = [ctypes.c_char_p]
        lib.axon_stop_nrt_profile.restype = ctypes.c_int64

        @contextlib.contextmanager
        def _hook(output_dir, device_ids):
            import jax

            jax.devices()
            if device_ids:
                ids = (ctypes.c_int64 * len(device_ids))(*device_ids)
                rc = lib.axon_start_nrt_profile(ids, len(device_ids))
            else:
                rc = lib.axon_start_nrt_profile(None, 0)
            if rc != 0:
                raise RuntimeError(f"axon_start_nrt_profile rc={rc}")
            try:
                yield
            finally:
                n = lib.axon_stop_nrt_profile(str(output_dir).encode())
                print(f"profile: {n} file(s) written to {output_dir}")

        mod.set_axon_ntff_profile_hook(_hook)
    except OSError:
        pass


_NC_CACHE = {}


def _build_nc():
    if "nc" in _NC_CACHE:
        return _NC_CACHE["nc"]
    from contextlib import ExitStack

    import concourse.bacc as bacc
    import concourse.mybir as mybir
    import concourse.tile as tile

    F32 = mybir.dt.float32
    F16 = mybir.dt.float16
    FP8 = mybir.dt.float8e4
    SUB = mybir.AluOpType.subtract
    DR = mybir.MatmulPerfMode.DoubleRow
    SQ = mybir.ActivationFunctionType.Square

    nc = bacc.Bacc("TRN2", target_bir_lowering=False, debug=False)
    # pre-transposed fp8 operands: [KP*128, 2*n] = [kp][dk][g][n]
    qt8 = nc.dram_tensor("qt8", [KP * P, 2 * QPC], FP8, kind="ExternalInput").ap()
    pt8 = nc.dram_tensor("pt8", [KP * P, 2 * PPC], FP8, kind="ExternalInput").ap()
    # f16 natural layouts for norms
    xq16 = nc.dram_tensor("xq16", [QPC, DIM], F16, kind="ExternalInput").ap()
    xp16 = nc.dram_tensor("xp16", [PPC, DIM], F16, kind="ExternalInput").ap()
    # col0 = sqrt(1/T) (ACT Square scale), col1 = 1/T (DVE square scalar),
    # precomputed on host, pre-broadcast to [128, 2]
    sinv_d = nc.dram_tensor("sinv", [P, 2], F32, kind="ExternalInput").ap()
    out = nc.dram_tensor("out", [QPC, PPC], F16, kind="ExternalOutput").ap()

    with tile.TileContext(nc) as tc:
        with ExitStack() as ctx:
            const = ctx.enter_context(tc.tile_pool(name="const", bufs=1))
            qpool = ctx.enter_context(tc.tile_pool(name="qpool", bufs=1))
            npool = ctx.enter_context(tc.tile_pool(name="npool", bufs=8))
            bpool = ctx.enter_context(tc.tile_pool(name="bpool", bufs=4))
            tpool = ctx.enter_context(tc.tile_pool(name="tpool", bufs=2))
            opool = ctx.enter_context(tc.tile_pool(name="opool", bufs=8))
            psum_mm = ctx.enter_context(
                tc.tile_pool(name="psum_mm", bufs=4, space="PSUM")
            )

            # ---- input DMAs ----
            # sync carries the fp8 operands + most naturals in need order;
            # scalar carries only two small early loads then the output DMAs;
            # gpsimd carries the psq gathers, bulk pt8 and later P naturals.
            sInv = const.tile([P, 2], F32)
            nc.sync.dma_start(sInv[:], sinv_d[:])

            # qts[kp][qq]: [128, 2, 512] = query columns qq*512..+512
            qts = [
                [
                    qpool.tile(
                        [P, 2, CH], FP8, tag=f"qt{kp}_{qq}", name=f"qt{kp}_{qq}"
                    )
                    for qq in range(4)
                ]
                for kp in range(KP)
            ]
            # pts[kp][cp]: [128, 2, 1024] = chunks 2cp, 2cp+1 of k-pair kp
            pts = [
                [
                    qpool.tile(
                        [P, 2, 2 * CH], FP8, tag=f"pt{kp}_{cp}", name=f"pt{kp}_{cp}"
                    )
                    for cp in range(NCH // 2)
                ]
                for kp in range(KP)
            ]
            qt8r = [
                qt8[kp * P : (kp + 1) * P, :].rearrange("p (g n) -> p g n", g=2)
                for kp in range(KP)
            ]
            pt8r = [
                pt8[kp * P : (kp + 1) * P, :].rearrange("p (g n) -> p g n", g=2)
                for kp in range(KP)
            ]
            # qnats[g]: [128, 4, 1024] f16 = query rows g*512..+512
            qnats = [
                qpool.tile([P, 4, DIM], F16, tag=f"qnat{g}", name=f"qnat{g}")
                for g in range(4)
            ]

            def dma_qt(qq, eng):
                for kp in range(KP):
                    eng.dma_start(
                        qts[kp][qq][:], qt8r[kp][:, :, qq * CH : (qq + 1) * CH]
                    )

            def dma_qnat(g, eng, split=1):
                # split engages multiple DMA engines (one DMA ~= one engine)
                rows = 512 // split
                for s in range(split):
                    eng.dma_start(
                        qnats[g][:, s * (4 // split) : (s + 1) * (4 // split), :],
                        xq16[
                            g * 512 + s * rows : g * 512 + (s + 1) * rows, :
                        ].rearrange("(i p) d -> p i d", p=P),
                    )

            def dma_pt(cp, eng, split=1):
                for kp in range(KP):
                    for s in range(split):
                        g0, g1 = s * (2 // split), (s + 1) * (2 // split)
                        eng.dma_start(
                            pts[kp][cp][:, g0:g1, :],
                            pt8r[kp][
                                :, g0:g1, cp * 2 * CH : (cp + 1) * 2 * CH
                            ],
                        )

            pnat_tiles = {}

            def dma_pnat(c, eng, split=1):
                # row c*512 + pp*4 + j -> pnat[pp, j, :], so psq4[pp, j] is
                # already in proto-column order for the flat [1, 512] gather
                pnat = npool.tile([P, CH // P, DIM], F16, tag="pnat", name="pnat")
                xpr = xp16[c * CH : (c + 1) * CH, :].rearrange(
                    "(p j) d -> p j d", p=P
                )
                for s in range(split):
                    j0, j1 = s * (4 // split), (s + 1) * (4 // split)
                    eng.dma_start(pnat[:, j0:j1, :], xpr[:, j0:j1, :])
                pnat_tiles[c] = pnat

            # scalar queue: first-pair fp8 operands (small), then output DMAs
            dma_qt(0, nc.scalar)
            dma_pt(0, nc.scalar, split=2)
            # sync: everything else, strictly in need order
            dma_pnat(0, nc.sync, split=2)
            dma_pnat(1, nc.sync, split=2)
            dma_qt(1, nc.sync)
            dma_qnat(0, nc.sync, split=2)
            dma_qnat(1, nc.sync)
            dma_qt(2, nc.sync)
            dma_qnat(2, nc.sync)
            dma_qt(3, nc.sync)
            dma_qnat(3, nc.sync)
            dma_pt(1, nc.sync, split=2)
            dma_pnat(2, nc.sync)
            dma_pnat(3, nc.sync)
            dma_pt(2, nc.sync)
            dma_pnat(4, nc.sync)
            dma_pnat(5, nc.sync)
            dma_pt(3, nc.sync)
            dma_pnat(6, nc.sync)
            dma_pnat(7, nc.sync)

            qsq = const.tile([P, NQT], F32)
            psq_b = qpool.tile([P, NCH, CH], F32, tag="psq_b")

            MULT = mybir.AluOpType.mult

            def q_squares(i, on_dve):
                trash = tpool.tile([P, DIM], F16, tag="trash", name="trash")
                if on_dve:
                    # DVE: (q * 1/T) * q summed = ||q||^2/T (f16 2x rate)
                    nc.vector.scalar_tensor_tensor(
                        out=trash[:],
                        in0=qnats[i // 4][:, i % 4, :],
                        scalar=sInv[:, 1:2],
                        in1=qnats[i // 4][:, i % 4, :],
                        op0=MULT,
                        op1=MULT,
                        accum_out=qsq[:, i : i + 1],
                    )
                else:
                    nc.scalar.activation(
                        out=trash[:],
                        in_=qnats[i // 4][:, i % 4, :],
                        func=SQ,
                        scale=sInv[:, 0:1],
                        accum_out=qsq[:, i : i + 1],
                    )

            def p_squares(c, dve_share):
                """||p||^2/T for chunk c -> broadcast tile psq_b[:, c, :]."""
                pnat = pnat_tiles.pop(c)
                psq4 = bpool.tile([P, CH // P], F32, tag="psq4", name="psq4")
                for j in range(4 - dve_share):
                    trash = tpool.tile([P, DIM], F16, tag="trash", name="trash")
                    nc.scalar.activation(
                        out=trash[:],
                        in_=pnat[:, j, :],
                        func=SQ,
                        scale=sInv[:, 0:1],
                        accum_out=psq4[:, j : j + 1],
                    )
                for j in range(4 - dve_share, 4):
                    trash = tpool.tile([P, DIM], F16, tag="trash", name="trash")
                    nc.vector.scalar_tensor_tensor(
                        out=trash[:],
                        in0=pnat[:, j, :],
                        scalar=sInv[:, 1:2],
                        in1=pnat[:, j, :],
                        op0=MULT,
                        op1=MULT,
                        accum_out=psq4[:, j : j + 1],
                    )
                psq_row = bpool.tile([1, CH], F32, tag="psq_row", name="psq_row")
                nc.gpsimd.dma_start(psq_row[:], psq4[:])
                nc.gpsimd.partition_broadcast(psq_b[:, c, :], psq_row[:])

            # early norms for the first chunk-pair, then the rest woven in.
            # gpsimd queue: gathers+broadcasts for c0/c1 go BEFORE the pt8
            # bulk loads so the first stt isn't stuck behind them.
            p_squares(0, dve_share=2)
            p_squares(1, dve_share=2)
            for i in range(4):
                q_squares(i, on_dve=True)



            # ---- matmul sweep over chunk pairs ----
            # q_squares(qb) is emitted just before pair0's stt needs qsq[qb];
            # p_squares for pair cp+1's chunks are woven into pair cp's sweep.
            for cp in range(NCH // 2):
                c0 = 2 * cp
                for qb in range(NQT):
                    if cp == 0 and qb >= 4:
                        q_squares(qb, on_dve=False)
                    if cp < NCH // 2 - 1:
                        if qb == 6:
                            p_squares(c0 + 2, dve_share=0)
                        elif qb == 12:
                            p_squares(c0 + 3, dve_share=0)
                    ps = psum_mm.tile([P, 2, CH], F32, tag="mm", name="mm")
                    for half in range(2):
                        for kp in range(KP):
                            nc.tensor.matmul(
                                ps[:, half, :],
                                qts[kp][qb // 4][
                                    :, :, (qb % 4) * P : (qb % 4 + 1) * P
                                ],
                                pts[kp][cp][:, :, half * CH : (half + 1) * CH],
                                start=(kp == 0),
                                stop=(kp == KP - 1),
                                perf_mode=DR,
                            )
                    ost = opool.tile([P, 2, CH], F16, tag="ost", name="ost")
                    nc.vector.scalar_tensor_tensor(
                        out=ost[:],
                        in0=ps[:],
                        scalar=qsq[:, qb : qb + 1],
                        in1=psq_b[:, c0 : c0 + 2, :],
                        op0=SUB,
                        op1=SUB,
                    )
                    if cp == NCH // 2 - 1:
                        # split the final pair's writes so the last transfer
                        # doesn't serialize an ~11us tail on one DMA engine
                        for s in range(2):
                            nc.scalar.dma_start(
                                out[
                                    qb * P : (qb + 1) * P,
                                    (c0 + s) * CH : (c0 + s + 1) * CH,
                                ],
                                ost[:, s, :],
                            )
                    else:
                        nc.scalar.dma_start(
                            out[qb * P : (qb + 1) * P, c0 * CH : (c0 + 2) * CH],
                            ost[:],
                        )

    nc.compile()
    _NC_CACHE["nc"] = nc
    return nc


def _host_stage(x, temperature):
    """Shard + marshal inputs: fp8 pre-transposed operands, f16 naturals."""
    import ml_dtypes

    e4 = ml_dtypes.float8_e4m3
    x = np.asarray(x, dtype=np.float32)
    t = np.asarray(temperature, dtype=np.float32).reshape(1, 1)
    s = np.float32(np.sqrt(2.0 / float(t[0, 0])))
    sinv = np.asarray([[np.sqrt(1.0 / float(t[0, 0]))]], dtype=np.float32)

    xq = x[:NUM_BATCH]
    xp = x[NUM_BATCH:]
    xq8 = (xq * s).astype(e4)
    xp8 = (xp * s).astype(e4)

    def packT(a8):
        # [n, 1024] fp8 -> [KP*128, 2*n] in [kp][dk][g][n] order
        tr = a8.T.reshape(KP, 2, P, a8.shape[0])
        return np.ascontiguousarray(tr.transpose(0, 2, 1, 3).reshape(KP * P, -1))

    qt8s = [packT(xq8[qs * QPC : (qs + 1) * QPC]) for qs in range(QSHARDS)]
    pt8s = [packT(xp8[p_ * PPC : (p_ + 1) * PPC]) for p_ in range(PSHARDS)]
    sc = np.asarray(
        [np.sqrt(1.0 / float(t[0, 0])), 1.0 / float(t[0, 0])], dtype=np.float32
    )
    sinv128 = np.ascontiguousarray(np.broadcast_to(sc, (P, 2)))
    xq16s = [
        np.ascontiguousarray(xq[qs * QPC : (qs + 1) * QPC].astype(np.float16))
        for qs in range(QSHARDS)
    ]
    xp16s = [
        np.ascontiguousarray(xp[p_ * PPC : (p_ + 1) * PPC].astype(np.float16))
        for p_ in range(PSHARDS)
    ]

    in_maps = []
    for c in range(N_CORES):
        qs, psh = divmod(c, PSHARDS)
        in_maps.append(
            {
                "qt8": qt8s[qs],
                "pt8": pt8s[psh],
                "xq16": xq16s[qs],
                "xp16": xp16s[psh],
                "sinv": sinv128,
            }
        )
    return in_maps


def _run(x, temperature, trace=False):
    _install_axon_hooks_shim()
    from concourse.bass_utils import run_bass_kernel_spmd

    nc = _build_nc()
    in_maps = _host_stage(x, temperature)
    res = run_bass_kernel_spmd(
        nc,
        in_maps,
        core_ids=list(range(N_CORES)),
        trace=trace,
        trace_cores=[0] if trace else None,
    )
    out = np.empty((NUM_BATCH, NUM_PROTO), dtype=np.float32)
    for c in range(N_CORES):
        qs, psh = divmod(c, PSHARDS)
        out[qs * QPC : (qs + 1) * QPC, psh * PPC : (psh + 1) * PPC] = res.results[
            c
        ]["out"].astype(np.float32)
    return out, res


def kernel(x, temperature, num_batch):
    assert int(num_batch) == NUM_BATCH, f"kernel hardcoded for num_batch={NUM_BATCH}"
    x = np.asarray(x)
    assert x.shape == (NUM_BATCH + NUM_PROTO, DIM), x.shape
    out, _ = _run(x, temperature, trace=False)
    return out
